# revision 6
# baseline (speedup 1.0000x reference)
"""Trainium2 Bass kernel for masked attention.

Reference semantics (B=4, S=4096, D=64):
    qs = q / 8
    scores = qs @ k.T + log(mask)[:, None, :]     # mask keys
    w = softmax(scores, axis=-1)
    out = w @ v
    return out * mask[..., None] + qs * (1 - mask)[..., None]

Sharding: 8 cores = (batch b = c//2, query half h = c%2). Each core
computes attention for 2048 queries of one batch with the batch's full
K/V/mask. Queries are independent -> no collectives.

Per-core pipeline (key-block-pair software pipeline, 4 query chunks):
  prep:  q/k loaded f32, converted to bf16 on GPSIMD (q duplicated, k in
         natural [pair, even|odd] layout), staged to DRAM scratch, then
         transposed DRAM->SBUF by the XBAR DMA-transpose engine. qT ends
         up replicated on both partition halves; kT has even key blocks
         on partitions 0:64 and odd blocks on 64:128. PE does no
         transposes in prep.
  QK:    two concurrent K=64 matmuls via PE row tiling
         (tile_position (0,0)/(64,0)) -> scoresT pair [128, 2, 512].
  exp:   E = exp(0.125*s - 2) in bf16, split across engines per pair:
         ACT pairs use the spline exp; DVE pairs use a Schraudolph
         bit-trick (one tensor_scalar: bits = s*A + B -> int16 = bf16).
         The -2 shift cancels in the softmax ratio.
  PV:    bf16 matmuls accumulate outT [65, 512]; Vaug = [mask*V, mask]
         so row 64 is the softmax denominator (no masking of E needed).
  epi:   PE transpose back per 128-query tile, divide by denominator,
         blend with qs passthrough for masked queries, DMA out.
"""

import numpy as np

import concourse.bacc as bacc
import concourse.bass as bass
import concourse.tile as tile
import concourse.mybir as mybir
from concourse.bass_utils import run_bass_kernel_spmd
from concourse.masks import make_identity

B, S, D = 4, 4096, 64
NCORES = 8
QSH = (B * S) // NCORES          # 2048 queries per core
NKB = S // 128                   # 32 key blocks
NPAIR = NKB // 2                 # 16 key-block pairs
NQT = QSH // 128                 # 16 query tiles
QCH = 512                        # query-chunk width
NQC = QSH // QCH                 # 4 query chunks
TPC = QCH // 128                 # 4 query tiles per chunk

F32 = mybir.dt.float32
BF16 = mybir.dt.bfloat16
I16 = mybir.dt.int16
Exp = mybir.ActivationFunctionType.Exp
MUL = mybir.AluOpType.mult
ADD = mybir.AluOpType.add

# E = exp(0.125*s - C_SHIFT); shift keeps ACT/Schraudolph outputs in a
# comfortable bf16 range and cancels between numerator and denominator.
C_SHIFT = 2.0
LOG2E = 1.4426950408889634
A_SCHR = 0.125 * LOG2E * 128                      # 23.0831...
B_SCHR = 128.0 * (127.0 - C_SHIFT * LOG2E) - 6.8  # bias-tuned Schraudolph
D_PAIRS = frozenset((1, 4, 7, 9, 12, 14, 15))     # pairs exp'd on DVE


def _emit(tc, nc, q_d, k_d, v_d, mk_d, mq_d, o_d):
    ctx_pools = []

    consts = tc.alloc_tile_pool(name="consts", bufs=1)
    sb = tc.alloc_tile_pool(name="sb", bufs=1)
    dscr = tc.alloc_tile_pool(name="dscr", bufs=1, space="DRAM")
    expp = tc.alloc_tile_pool(name="expp", bufs=3)
    otp = tc.alloc_tile_pool(name="otp", bufs=2)
    finp = tc.alloc_tile_pool(name="finp", bufs=2)
    ctx_pools += [consts, sb, dscr, expp, otp, finp]

    identity = consts.tile([128, 128], F32, name="identity")
    make_identity(nc, identity)
    # warm the ACT exp table before the pipeline needs it
    actwarm = consts.tile([1, 1], F32, name="actwarm")
    nc.scalar.activation(out=actwarm, in_=identity[0:1, 0:1], func=Exp)
    bshift = consts.tile([128, 1], F32, name="bshift")
    nc.gpsimd.memset(bshift[:, :], -C_SHIFT)

    q3 = sb.tile([128, NQT, D], F32, name="q3")         # q3[p,n] = q row p*16+n
    k3 = sb.tile([128, NPAIR, 2, D], F32, name="k3")    # k3[p,i,j] = k row p*32+2i+j
    v3f = sb.tile([128, NKB, D], F32, name="v3f")
    mk = sb.tile([128, NKB], F32, name="mk_sb")
    mq = sb.tile([128, NQT], F32, name="mq_sb")
    s1 = sb.tile([128, NQT], F32, name="s1_sb")         # 0.125*(1-mq)
    qb3 = sb.tile([128, NQT, D], F32, name="qb3")       # qs*(1-mq) passthrough
    q3bd = sb.tile([128, NQT, 2, D], BF16, name="q3bd")  # bf16 q, duplicated
    k3b = sb.tile([128, NPAIR, 2, D], BF16, name="k3b")  # bf16 k, pair layout
    qTd = sb.tile([128, NQT, 128], BF16, name="qTd")    # qT on both halves
    kTd = sb.tile([128, NPAIR, 128], BF16, name="kTd")  # lo=even kb, hi=odd kb
    vb = sb.tile([128, NKB, D + 1], BF16, name="vb")    # [mask*V, mask]

    q_scr = dscr.tile([NQT * 128, 128], BF16, name="q_scr")
    k_scr = dscr.tile([NPAIR * 128, 128], BF16, name="k_scr")

    qap = q_d.ap().rearrange("(p n) d -> p n d", p=128)
    kap = k_d.ap().rearrange("(p n) d -> p n d", p=128)
    vap = v_d.ap().rearrange("(p n) d -> p n d", p=128)
    oap = o_d.ap().rearrange("(p n) d -> p n d", p=128)

    def stage_q(t0, t1):
        nc.sync.dma_start(
            out=q_scr[128 * t0:128 * t1, :].rearrange("(i p) c -> p i c", p=128),
            in_=q3bd[:, t0:t1, :, :].rearrange("p i j d -> p i (j d)"))

    def stage_k(i0, i1):
        nc.sync.dma_start(
            out=k_scr[128 * i0:128 * i1, :].rearrange("(i p) c -> p i c", p=128),
            in_=k3b[:, i0:i1, :, :].rearrange("p i j d -> p i (j d)"))

    def xpose_q(t0, t1):
        nc.sync.dma_start_transpose(
            qTd[:, t0:t1, :].rearrange("p i c -> p (i c)"),
            q_scr[128 * t0:128 * t1, :])

    def xpose_k(i0, i1):
        nc.sync.dma_start_transpose(
            kTd[:, i0:i1, :].rearrange("p i c -> p (i c)"),
            k_scr[128 * i0:128 * i1, :])

    def conv_q(t0, t1):
        nc.gpsimd.tensor_copy(q3bd[:, t0:t1, 0, :], q3[:, t0:t1, :])
        nc.gpsimd.tensor_copy(q3bd[:, t0:t1, 1, :], q3[:, t0:t1, :])

    # ---- startup: chunk 0 queries + key pairs 0-1 first ----
    nc.sync.dma_start(out=q3[:, 0:4, :], in_=qap[:, 0:4, :])
    nc.sync.dma_start(out=k3[:, 0:2, :, :], in_=kap[:, 0:4, :])
    nc.sync.dma_start(out=mk, in_=mk_d.ap().rearrange("(p n) -> p n", p=128))
    nc.sync.dma_start(out=mq, in_=mq_d.ap().rearrange("(p n) -> p n", p=128))
    nc.sync.dma_start(out=v3f[:, 0:8, :], in_=vap[:, 0:8, :])

    conv_q(0, 4)
    nc.gpsimd.tensor_copy(k3b[:, 0:2, :, :], k3[:, 0:2, :, :])
    stage_q(0, 4)
    stage_k(0, 2)
    xpose_q(0, 4)
    xpose_k(0, 2)
    # first PV weights + denominator column for all blocks (needs only mk)
    nc.gpsimd.tensor_copy(vb[:, :, D:D + 1], mk.rearrange("p (n o) -> p n o", o=1))
    for kb in range(4):
        nc.gpsimd.tensor_scalar_mul(vb[:, kb, 0:D], v3f[:, kb, :], mk[:, kb:kb + 1])

    # ---- remaining loads ----
    nc.sync.dma_start(out=k3[:, 2:NPAIR, :, :], in_=kap[:, 4:NKB, :])
    nc.sync.dma_start(out=q3[:, 4:NQT, :], in_=qap[:, 4:NQT, :])
    for g in range(1, 4):
        nc.sync.dma_start(out=v3f[:, 8 * g:8 * g + 8, :], in_=vap[:, 8 * g:8 * g + 8, :])

    # ---- remaining converts + stages + transposes, pair-ordered ----
    nc.gpsimd.tensor_copy(k3b[:, 2:8, :, :], k3[:, 2:8, :, :])
    stage_k(2, 8)
    xpose_k(2, 8)
    for kb in range(4, 8):
        nc.gpsimd.tensor_scalar_mul(vb[:, kb, 0:D], v3f[:, kb, :], mk[:, kb:kb + 1])
    nc.gpsimd.tensor_copy(k3b[:, 8:NPAIR, :, :], k3[:, 8:NPAIR, :, :])
    stage_k(8, NPAIR)
    xpose_k(8, NPAIR)
    conv_q(4, NQT)
    stage_q(4, NQT)
    xpose_q(4, NQT)
    for kb in range(8, NKB):
        nc.gpsimd.tensor_scalar_mul(vb[:, kb, 0:D], v3f[:, kb, :], mk[:, kb:kb + 1])

    # passthrough term, off the critical path
    nc.gpsimd.tensor_scalar(s1, mq, -0.125, 0.125, MUL, ADD)
    for qt in range(NQT):
        nc.gpsimd.tensor_scalar_mul(qb3[:, qt, :], q3[:, qt, :], s1[:, qt:qt + 1])

    # ---- main loop ----
    ps_sc = tc.alloc_tile_pool(name="ps_sc", bufs=2, space="PSUM")
    ps_o = tc.alloc_tile_pool(name="ps_o", bufs=2, space="PSUM")
    ps_e = tc.alloc_tile_pool(name="ps_e", bufs=1, space="PSUM")
    ctx_pools += [ps_sc, ps_o, ps_e]

    def qk(qc, i):
        sc = ps_sc.tile([128, 2, QCH], F32, name=f"sc{qc}_{i}", tag="sc")
        nc.tensor.matmul(sc[:, 0, :], lhsT=kTd[0:64, i, :],
                         rhs=qTd[0:64, 4 * qc:4 * qc + 4, :],
                         start=True, stop=True, tile_position=(0, 0))
        nc.tensor.matmul(sc[:, 1, :], lhsT=kTd[64:128, i, :],
                         rhs=qTd[64:128, 4 * qc:4 * qc + 4, :],
                         start=True, stop=True, tile_position=(64, 0))
        return sc

    def emit_epilogue(qc, oT_ps):
        oT_sb = otp.tile([D + 1, QCH], F32, name=f"oT_sb{qc}", tag="otsb")
        nc.scalar.copy(out=oT_sb, in_=oT_ps)
        tp = ps_e.tile([128, TPC, D + 1], F32, name=f"tp{qc}", tag="tp")
        for t in range(TPC):
            nc.tensor.transpose(tp[:, t, :], oT_sb[:, 128 * t:128 * (t + 1)],
                                identity[0:D + 1, 0:D + 1])
        rec = finp.tile([128, TPC], F32, name=f"rec{qc}", tag="rec")
        nc.vector.reciprocal(rec, tp[:, :, D:D + 1])
        recm = finp.tile([128, TPC], F32, name=f"recm{qc}", tag="recm")
        nc.vector.tensor_tensor(recm, rec, mq[:, 4 * qc:4 * qc + 4], MUL)
        fin3 = finp.tile([128, TPC, D], F32, name=f"fin3_{qc}", tag="fin3")
        for t in range(TPC):
            nc.vector.scalar_tensor_tensor(fin3[:, t, :], tp[:, t, 0:D],
                                           recm[:, t:t + 1], qb3[:, 4 * qc + t, :],
                                           MUL, ADD)
        nc.sync.dma_start(out=oap[:, 4 * qc:4 * qc + 4, :], in_=fin3)

    pend_epi = None
    for qc in range(NQC):
        oT_ps = ps_o.tile([D + 1, QCH], F32, name=f"oT_ps{qc}", tag="ot")
        scs = {0: qk(qc, 0), 1: qk(qc, 1)}
        if pend_epi is not None:
            emit_epilogue(*pend_epi)
            pend_epi = None
        for i in range(NPAIR):
            sc = scs.pop(i)
            ex = expp.tile([128, 2, QCH], BF16, name=f"ex{qc}_{i}", tag="ex")
            if i in D_PAIRS:
                nc.vector.tensor_scalar(ex[:, :, :].bitcast(I16), sc[:, :, :],
                                        A_SCHR, B_SCHR, MUL, ADD)
            else:
                nc.scalar.activation(out=ex, in_=sc[:, :, :], func=Exp,
                                     scale=0.125, bias=bshift[:, 0:1])
            if i + 2 < NPAIR:
                scs[i + 2] = qk(qc, i + 2)
            nc.tensor.matmul(oT_ps, lhsT=vb[:, 2 * i, :], rhs=ex[:, 0, :],
                             start=(i == 0), stop=False)
            nc.tensor.matmul(oT_ps, lhsT=vb[:, 2 * i + 1, :], rhs=ex[:, 1, :],
                             start=False, stop=(i == NPAIR - 1))
        pend_epi = (qc, oT_ps)
    emit_epilogue(*pend_epi)

    for p in reversed(ctx_pools):
        p.release()


_PROGS = {}


def _build(repeat=1, loop=None, ablate=()):
    key = (repeat, loop, tuple(ablate))
    if key in _PROGS:
        return _PROGS[key]
    nc = bacc.Bacc("TRN2", target_bir_lowering=False, debug=False)
    q_d = nc.dram_tensor("q_in", [QSH, D], F32, kind="ExternalInput")
    k_d = nc.dram_tensor("k_in", [S, D], F32, kind="ExternalInput")
    v_d = nc.dram_tensor("v_in", [S, D], F32, kind="ExternalInput")
    mk_d = nc.dram_tensor("mk_in", [S], F32, kind="ExternalInput")
    mq_d = nc.dram_tensor("mq_in", [QSH], F32, kind="ExternalInput")
    o_d = nc.dram_tensor("o_out", [QSH, D], F32, kind="ExternalOutput")
    with tile.TileContext(nc) as tc:
        if loop is not None:
            with tc.For_i(0, loop, 1):
                for _ in range(repeat):
                    _emit(tc, nc, q_d, k_d, v_d, mk_d, mq_d, o_d)
        else:
            for _ in range(repeat):
                _emit(tc, nc, q_d, k_d, v_d, mk_d, mq_d, o_d)
    nc.compile()
    _PROGS[key] = nc
    return nc


def make_in_maps(q, k, v, mask):
    q = np.ascontiguousarray(np.asarray(q, dtype=np.float32))
    k = np.ascontiguousarray(np.asarray(k, dtype=np.float32))
    v = np.ascontiguousarray(np.asarray(v, dtype=np.float32))
    mask = np.ascontiguousarray(np.asarray(mask, dtype=np.float32))
    in_maps = []
    for c in range(NCORES):
        b, h = c // 2, c % 2
        sl = slice(h * QSH, (h + 1) * QSH)
        in_maps.append({
            "q_in": np.ascontiguousarray(q[b, sl, :]),
            "k_in": np.ascontiguousarray(k[b]),
            "v_in": np.ascontiguousarray(v[b]),
            "mk_in": np.ascontiguousarray(mask[b]),
            "mq_in": np.ascontiguousarray(mask[b, sl]),
        })
    return in_maps


def gather(results):
    out = np.empty((B, S, D), np.float32)
    for c in range(NCORES):
        b, h = c // 2, c % 2
        out[b, h * QSH:(h + 1) * QSH, :] = results[c]["o_out"]
    return out


def kernel(q, k, v, mask, _spmd_kwargs=None):
    nc = _build()
    in_maps = make_in_maps(q, k, v, mask)
    res = run_bass_kernel_spmd(nc, in_maps, core_ids=list(range(NCORES)),
                               **(_spmd_kwargs or {}))
    out = gather(res.results)
    if _spmd_kwargs:
        kernel._last_results = res
    return out


# revision 21
# speedup vs baseline: 1.1005x; 1.1005x over previous
"""Trainium2 Bass kernel for masked attention.

Reference semantics (B=4, S=4096, D=64):
    qs = q / 8
    scores = qs @ k.T + log(mask)[:, None, :]     # mask keys
    w = softmax(scores, axis=-1)
    out = w @ v
    return out * mask[..., None] + qs * (1 - mask)[..., None]

Sharding: 8 cores = (batch b = c//2, query half h = c%2). Each core
computes attention for 2048 queries of one batch with the batch's full
K/V/mask. Queries are independent -> no collectives.

Per-core pipeline (key-block-pair software pipeline, 4 query chunks):
  prep:  q/k loaded f32, converted to bf16 on GPSIMD (q duplicated, k in
         natural [pair, even|odd] layout), staged to DRAM scratch, then
         transposed DRAM->SBUF by the XBAR DMA-transpose engine. qT ends
         up replicated on both partition halves; kT has even key blocks
         on partitions 0:64 and odd blocks on 64:128. PE does no
         transposes in prep.
  QK:    two concurrent K=64 matmuls via PE row tiling
         (tile_position (0,0)/(64,0)) -> scoresT pair [128, 2, 512].
  exp:   E = exp(0.125*s - 2) in bf16, split across engines per pair:
         ACT pairs use the spline exp; DVE pairs use a Schraudolph
         bit-trick (one tensor_scalar: bits = s*A + B -> int16 = bf16).
         The -2 shift cancels in the softmax ratio.
  PV:    bf16 matmuls accumulate outT [65, 512]; Vaug = [mask*V, mask]
         so row 64 is the softmax denominator (no masking of E needed).
  epi:   PE transpose back per 128-query tile, divide by denominator,
         blend with qs passthrough for masked queries, DMA out.
"""

import numpy as np

import concourse.bacc as bacc
import concourse.bass as bass
import concourse.tile as tile
import concourse.mybir as mybir
from concourse.bass_utils import run_bass_kernel_spmd
from concourse.masks import make_identity

B, S, D = 4, 4096, 64
NCORES = 8
QSH = (B * S) // NCORES          # 2048 queries per core
NKB = S // 128                   # 32 key blocks
NPAIR = NKB // 2                 # 16 key-block pairs
NQT = QSH // 128                 # 16 query tiles
QCH = 512                        # query-chunk width
NQC = QSH // QCH                 # 4 query chunks
TPC = QCH // 128                 # 4 query tiles per chunk

F32 = mybir.dt.float32
BF16 = mybir.dt.bfloat16
I16 = mybir.dt.int16
Exp = mybir.ActivationFunctionType.Exp
MUL = mybir.AluOpType.mult
ADD = mybir.AluOpType.add

# E = exp(0.125*s - C_SHIFT); shift keeps ACT/Schraudolph outputs in a
# comfortable bf16 range and cancels between numerator and denominator.
C_SHIFT = 2.0
LOG2E = 1.4426950408889634
A_SCHR = 0.125 * LOG2E * 128                      # 23.0831...
B_SCHR = 128.0 * (127.0 - C_SHIFT * LOG2E) - 6.8  # bias-tuned Schraudolph
# Pairs exp'd on DVE, per chunk. Chunk 0 leans on ACT while DVE finishes
# the bf16 prep converts; later chunks split ~9 ACT / 7 DVE.
D_PAIRS = (
    frozenset((9, 11, 13, 15)),
    frozenset((1, 4, 7, 9, 12, 14, 15)),
    frozenset((1, 4, 7, 9, 12, 14, 15)),
    frozenset((1, 4, 7, 9, 12, 14, 15)),
)


def _emit(tc, nc, q_d, k_d, v_d, mk_d, mq_d, o_d):
    ctx_pools = []

    consts = tc.alloc_tile_pool(name="consts", bufs=1)
    sb = tc.alloc_tile_pool(name="sb", bufs=1)
    dscr = tc.alloc_tile_pool(name="dscr", bufs=1, space="DRAM")
    expp = tc.alloc_tile_pool(name="expp", bufs=3)
    otp = tc.alloc_tile_pool(name="otp", bufs=2)
    finp = tc.alloc_tile_pool(name="finp", bufs=2)
    ctx_pools += [consts, sb, dscr, expp, otp, finp]

    identity = consts.tile([128, 128], F32, name="identity")
    make_identity(nc, identity)
    # warm the ACT exp table before the pipeline needs it
    actwarm = consts.tile([1, 1], F32, name="actwarm")
    nc.scalar.activation(out=actwarm, in_=identity[0:1, 0:1], func=Exp)
    bshift = consts.tile([128, 1], F32, name="bshift")
    nc.gpsimd.memset(bshift[:, :], -C_SHIFT)

    q3 = sb.tile([128, NQT, D], F32, name="q3")         # q3[p,n] = q row p*16+n
    k3 = sb.tile([128, NPAIR, 2, D], F32, name="k3")    # k3[p,i,j] = k row p*32+2i+j
    v3f = sb.tile([128, NKB, D], F32, name="v3f")
    mk = sb.tile([128, NKB], F32, name="mk_sb")
    mq = sb.tile([128, NQT], F32, name="mq_sb")
    s1 = sb.tile([128, NQT], F32, name="s1_sb")         # 0.125*(1-mq)
    qb3 = sb.tile([128, NQT, D], F32, name="qb3")       # qs*(1-mq) passthrough
    # Staged tensors are split into per-stage tiles: the Tile framework
    # tracks hazards per tile, so a single big tile would serialize early
    # consumers behind late producers.
    q3bd0 = sb.tile([128, 4, 2, D], BF16, name="q3bd0")   # bf16 q, duplicated
    q3bdR = sb.tile([128, NQT - 4, 2, D], BF16, name="q3bdR")
    k3b_a = sb.tile([128, 6, 2, D], BF16, name="k3b_a")   # bf16 k, pair layout
    k3b_b = sb.tile([128, 5, 2, D], BF16, name="k3b_b")
    k3b_c = sb.tile([128, 5, 2, D], BF16, name="k3b_c")
    qTd0 = sb.tile([128, 4, 128], BF16, name="qTd0")      # qT on both halves
    qTdR = sb.tile([128, NQT - 4, 128], BF16, name="qTdR")
    kTd_a = sb.tile([128, 6, 128], BF16, name="kTd_a")    # lo=even, hi=odd kb
    kTd_b = sb.tile([128, 5, 128], BF16, name="kTd_b")
    kTd_c = sb.tile([128, 5, 128], BF16, name="kTd_c")
    # [mask*V, mask] per 4 key blocks
    vb8 = [sb.tile([128, 4, D + 1], BF16, name=f"vb{g}") for g in range(NKB // 4)]

    q_scr0 = dscr.tile([4 * 128, 128], BF16, name="q_scr0")
    q_scrR = dscr.tile([(NQT - 4) * 128, 128], BF16, name="q_scrR")
    k_scr_a = dscr.tile([6 * 128, 128], BF16, name="k_scr_a")
    k_scr_b = dscr.tile([5 * 128, 128], BF16, name="k_scr_b")
    k_scr_c = dscr.tile([5 * 128, 128], BF16, name="k_scr_c")

    def kt_sel(i):
        if i < 6:
            return kTd_a, i
        if i < 11:
            return kTd_b, i - 6
        return kTd_c, i - 11

    def vb_sel(kb):
        return vb8[kb // 4][:, kb % 4, :]

    qap = q_d.ap().rearrange("(p n) d -> p n d", p=128)
    kap = k_d.ap().rearrange("(p n) d -> p n d", p=128)
    vap = v_d.ap().rearrange("(p n) d -> p n d", p=128)
    oap = o_d.ap().rearrange("(p n) d -> p n d", p=128)

    def stage(eng, scr, b4):
        eng.dma_start(
            out=scr[:, :].rearrange("(i p) c -> p i c", p=128),
            in_=b4[:, :, :, :].rearrange("p i j d -> p i (j d)"))

    def xpose(eng, td, scr):
        eng.dma_start_transpose(
            td[:, :, :].rearrange("p i c -> p (i c)"), scr[:, :])

    # ---- startup loads. Two parallel HWDGE rings: the q chain rides the
    # SP ring, the k chain rides the ACT ring; bulk loads go through the
    # GPSIMD SWDGE path so they never block either transpose chain. ----
    nc.sync.dma_start(out=mk, in_=mk_d.ap().rearrange("(p n) -> p n", p=128))
    nc.sync.dma_start(out=mq, in_=mq_d.ap().rearrange("(p n) -> p n", p=128))
    nc.sync.dma_start(out=q3[:, 0:4, :], in_=qap[:, 0:4, :])
    nc.scalar.dma_start(out=k3[:, 0:6, :, :], in_=kap[:, 0:12, :])

    # bulk loads + vb prep on GPSIMD, in consumption order
    nc.gpsimd.dma_start(out=v3f[:, 0:4, :], in_=vap[:, 0:4, :])
    # startup converts: q on Pool, k on DVE (both otherwise idle here)
    nc.gpsimd.tensor_copy(q3bd0[:, :, 0, :], q3[:, 0:4, :])
    nc.gpsimd.tensor_copy(q3bd0[:, :, 1, :], q3[:, 0:4, :])
    nc.vector.tensor_copy(k3b_a, k3[:, 0:6, :, :])
    stage(nc.sync, q_scr0, q3bd0)
    stage(nc.sync, k_scr_a, k3b_a)
    xpose(nc.sync, qTd0, q_scr0)
    xpose(nc.sync, kTd_a, k_scr_a)

    nc.gpsimd.dma_start(out=v3f[:, 4:8, :], in_=vap[:, 4:8, :])
    for g in range(NKB // 4):
        nc.gpsimd.tensor_copy(vb8[g][:, :, D:D + 1],
                              mk[:, 4 * g:4 * g + 4].rearrange("p (n o) -> p n o", o=1))
    for kb in range(4):
        nc.gpsimd.tensor_scalar_mul(vb_sel(kb)[:, 0:D], v3f[:, kb, :], mk[:, kb:kb + 1])
    nc.gpsimd.dma_start(out=k3[:, 6:NPAIR, :, :], in_=kap[:, 12:NKB, :])
    for kb in range(4, 8):
        nc.gpsimd.tensor_scalar_mul(vb_sel(kb)[:, 0:D], v3f[:, kb, :], mk[:, kb:kb + 1])
    nc.gpsimd.dma_start(out=v3f[:, 8:20, :], in_=vap[:, 8:20, :])
    nc.gpsimd.dma_start(out=q3[:, 4:NQT, :], in_=qap[:, 4:NQT, :])

    # ---- remaining converts (DVE) + stages + transposes, pair-ordered ----
    nc.vector.tensor_copy(k3b_b, k3[:, 6:11, :, :])
    stage(nc.sync, k_scr_b, k3b_b)
    xpose(nc.sync, kTd_b, k_scr_b)
    for kb in range(8, 16):
        nc.gpsimd.tensor_scalar_mul(vb_sel(kb)[:, 0:D], v3f[:, kb, :], mk[:, kb:kb + 1])
    nc.gpsimd.dma_start(out=v3f[:, 20:NKB, :], in_=vap[:, 20:NKB, :])
    nc.vector.tensor_copy(k3b_c, k3[:, 11:NPAIR, :, :])
    stage(nc.sync, k_scr_c, k3b_c)
    xpose(nc.sync, kTd_c, k_scr_c)
    nc.vector.tensor_copy(q3bdR[:, :, 0, :], q3[:, 4:NQT, :])
    nc.vector.tensor_copy(q3bdR[:, :, 1, :], q3[:, 4:NQT, :])
    stage(nc.sync, q_scrR, q3bdR)
    xpose(nc.sync, qTdR, q_scrR)

    # rest of PV weights + passthrough term on GPSIMD, pair-ordered
    for kb in range(16, NKB):
        nc.gpsimd.tensor_scalar_mul(vb_sel(kb)[:, 0:D], v3f[:, kb, :], mk[:, kb:kb + 1])
    nc.gpsimd.tensor_scalar(s1, mq, -0.125, 0.125, MUL, ADD)
    for qt in range(NQT):
        nc.gpsimd.tensor_scalar_mul(qb3[:, qt, :], q3[:, qt, :], s1[:, qt:qt + 1])

    # ---- main loop ----
    ps_sc = tc.alloc_tile_pool(name="ps_sc", bufs=3, space="PSUM")
    ps_o = tc.alloc_tile_pool(name="ps_o", bufs=1, space="PSUM")
    ps_e = tc.alloc_tile_pool(name="ps_e", bufs=1, space="PSUM")
    ctx_pools += [ps_sc, ps_o, ps_e]

    def qk(qc, i):
        kta, il = kt_sel(i)
        qta, q0 = (qTd0, 0) if qc == 0 else (qTdR, 4 * (qc - 1))
        sc = ps_sc.tile([128, 2, QCH], F32, name=f"sc{qc}_{i}", tag="sc")
        nc.tensor.matmul(sc[:, 0, :], lhsT=kta[0:64, il, :],
                         rhs=qta[0:64, q0:q0 + 4, :],
                         start=True, stop=True, tile_position=(0, 0))
        nc.tensor.matmul(sc[:, 1, :], lhsT=kta[64:128, il, :],
                         rhs=qta[64:128, q0:q0 + 4, :],
                         start=True, stop=True, tile_position=(64, 0))
        return sc

    def emit_epilogue(qc, oT_ps):
        oT_sb = otp.tile([D + 1, QCH], F32, name=f"oT_sb{qc}", tag="otsb")
        nc.scalar.copy(out=oT_sb, in_=oT_ps)
        tp = ps_e.tile([128, TPC, D + 1], F32, name=f"tp{qc}", tag="tp")
        for t in range(TPC):
            nc.tensor.transpose(tp[:, t, :], oT_sb[:, 128 * t:128 * (t + 1)],
                                identity[0:D + 1, 0:D + 1])
        rec = finp.tile([128, TPC], F32, name=f"rec{qc}", tag="rec")
        nc.vector.reciprocal(rec, tp[:, :, D:D + 1])
        recm = finp.tile([128, TPC], F32, name=f"recm{qc}", tag="recm")
        nc.vector.tensor_tensor(recm, rec, mq[:, 4 * qc:4 * qc + 4], MUL)
        fin3 = finp.tile([128, TPC, D], F32, name=f"fin3_{qc}", tag="fin3")
        for t in range(TPC):
            nc.vector.scalar_tensor_tensor(fin3[:, t, :], tp[:, t, 0:D],
                                           recm[:, t:t + 1], qb3[:, 4 * qc + t, :],
                                           MUL, ADD)
        nc.sync.dma_start(out=oap[:, 4 * qc:4 * qc + 4, :], in_=fin3)

    pend_epi = None
    for qc in range(NQC):
        oT_ps = ps_o.tile([D + 1, QCH], F32, name=f"oT_ps{qc}", tag="ot")
        scs = {i: qk(qc, i) for i in range(3)}
        if pend_epi is not None:
            emit_epilogue(*pend_epi)
            pend_epi = None
        for i in range(NPAIR):
            sc = scs.pop(i)
            ex = expp.tile([128, 2, QCH], BF16, name=f"ex{qc}_{i}", tag="ex")
            if i in D_PAIRS[qc]:
                nc.vector.tensor_scalar(ex[:, :, :].bitcast(I16), sc[:, :, :],
                                        A_SCHR, B_SCHR, MUL, ADD)
            else:
                nc.scalar.activation(out=ex, in_=sc[:, :, :], func=Exp,
                                     scale=0.125, bias=bshift[:, 0:1])
            if i + 3 < NPAIR:
                scs[i + 3] = qk(qc, i + 3)
            nc.tensor.matmul(oT_ps, lhsT=vb_sel(2 * i), rhs=ex[:, 0, :],
                             start=(i == 0), stop=False)
            nc.tensor.matmul(oT_ps, lhsT=vb_sel(2 * i + 1), rhs=ex[:, 1, :],
                             start=False, stop=(i == NPAIR - 1))
        pend_epi = (qc, oT_ps)
    emit_epilogue(*pend_epi)

    for p in reversed(ctx_pools):
        p.release()


_PROGS = {}


def _build(repeat=1, loop=None, ablate=()):
    key = (repeat, loop, tuple(ablate))
    if key in _PROGS:
        return _PROGS[key]
    nc = bacc.Bacc("TRN2", target_bir_lowering=False, debug=False)
    q_d = nc.dram_tensor("q_in", [QSH, D], F32, kind="ExternalInput")
    k_d = nc.dram_tensor("k_in", [S, D], F32, kind="ExternalInput")
    v_d = nc.dram_tensor("v_in", [S, D], F32, kind="ExternalInput")
    mk_d = nc.dram_tensor("mk_in", [S], F32, kind="ExternalInput")
    mq_d = nc.dram_tensor("mq_in", [QSH], F32, kind="ExternalInput")
    o_d = nc.dram_tensor("o_out", [QSH, D], F32, kind="ExternalOutput")
    with tile.TileContext(nc) as tc:
        if loop is not None:
            with tc.For_i(0, loop, 1):
                for _ in range(repeat):
                    _emit(tc, nc, q_d, k_d, v_d, mk_d, mq_d, o_d)
        else:
            for _ in range(repeat):
                _emit(tc, nc, q_d, k_d, v_d, mk_d, mq_d, o_d)
    nc.compile()
    _PROGS[key] = nc
    return nc


def make_in_maps(q, k, v, mask):
    q = np.ascontiguousarray(np.asarray(q, dtype=np.float32))
    k = np.ascontiguousarray(np.asarray(k, dtype=np.float32))
    v = np.ascontiguousarray(np.asarray(v, dtype=np.float32))
    mask = np.ascontiguousarray(np.asarray(mask, dtype=np.float32))
    in_maps = []
    for c in range(NCORES):
        b, h = c // 2, c % 2
        sl = slice(h * QSH, (h + 1) * QSH)
        in_maps.append({
            "q_in": np.ascontiguousarray(q[b, sl, :]),
            "k_in": np.ascontiguousarray(k[b]),
            "v_in": np.ascontiguousarray(v[b]),
            "mk_in": np.ascontiguousarray(mask[b]),
            "mq_in": np.ascontiguousarray(mask[b, sl]),
        })
    return in_maps


def gather(results):
    out = np.empty((B, S, D), np.float32)
    for c in range(NCORES):
        b, h = c // 2, c % 2
        out[b, h * QSH:(h + 1) * QSH, :] = results[c]["o_out"]
    return out


def kernel(q, k, v, mask, _spmd_kwargs=None):
    nc = _build()
    in_maps = make_in_maps(q, k, v, mask)
    res = run_bass_kernel_spmd(nc, in_maps, core_ids=list(range(NCORES)),
                               **(_spmd_kwargs or {}))
    out = gather(res.results)
    if _spmd_kwargs:
        kernel._last_results = res
    return out


# revision 23
# speedup vs baseline: 1.2745x; 1.1581x over previous
"""Trainium2 Bass kernel for masked attention.

Reference semantics (B=4, S=4096, D=64):
    qs = q / 8
    scores = qs @ k.T + log(mask)[:, None, :]     # mask keys
    w = softmax(scores, axis=-1)
    out = w @ v
    return out * mask[..., None] + qs * (1 - mask)[..., None]

Sharding: 8 cores = (batch b = c//2, query half h = c%2). Each core
computes attention for 2048 queries of one batch with the batch's full
K/V/mask. Queries are independent -> no collectives.

Per-core pipeline (key-block-pair software pipeline, 4 query chunks):
  prep:  q/k loaded f32, converted to bf16 on GPSIMD (q duplicated, k in
         natural [pair, even|odd] layout), staged to DRAM scratch, then
         transposed DRAM->SBUF by the XBAR DMA-transpose engine. qT ends
         up replicated on both partition halves; kT has even key blocks
         on partitions 0:64 and odd blocks on 64:128. PE does no
         transposes in prep.
  QK:    two concurrent K=64 matmuls via PE row tiling
         (tile_position (0,0)/(64,0)) -> scoresT pair [128, 2, 512].
  exp:   E = exp(0.125*s - 2) in bf16, split across engines per pair:
         ACT pairs use the spline exp; DVE pairs use a Schraudolph
         bit-trick (one tensor_scalar: bits = s*A + B -> int16 = bf16).
         The -2 shift cancels in the softmax ratio.
  PV:    bf16 matmuls accumulate outT [65, 512]; Vaug = [mask*V, mask]
         so row 64 is the softmax denominator (no masking of E needed).
  epi:   PE transpose back per 128-query tile, divide by denominator,
         blend with qs passthrough for masked queries, DMA out.
"""

import numpy as np

import concourse.bacc as bacc
import concourse.bass as bass
import concourse.tile as tile
import concourse.mybir as mybir
from concourse.bass_utils import run_bass_kernel_spmd
from concourse.masks import make_identity

B, S, D = 4, 4096, 64
NCORES = 8
QSH = (B * S) // NCORES          # 2048 queries per core
NKB = S // 128                   # 32 key blocks
NPAIR = NKB // 2                 # 16 key-block pairs
NQT = QSH // 128                 # 16 query tiles
QCH = 512                        # query-chunk width
NQC = QSH // QCH                 # 4 query chunks
TPC = QCH // 128                 # 4 query tiles per chunk

F32 = mybir.dt.float32
BF16 = mybir.dt.bfloat16
I16 = mybir.dt.int16
Exp = mybir.ActivationFunctionType.Exp
MUL = mybir.AluOpType.mult
ADD = mybir.AluOpType.add

# E = exp(0.125*s - C_SHIFT); shift keeps ACT/Schraudolph outputs in a
# comfortable bf16 range and cancels between numerator and denominator.
C_SHIFT = 2.0
LOG2E = 1.4426950408889634
A_SCHR = 0.125 * LOG2E * 128                      # 23.0831...
B_SCHR = 128.0 * (127.0 - C_SHIFT * LOG2E) - 6.8  # bias-tuned Schraudolph
# Pairs exp'd on DVE, per chunk. Chunk 0 leans on ACT while DVE finishes
# the bf16 prep converts; later chunks split ~9 ACT / 7 DVE.
D_PAIRS = (
    frozenset((9, 11, 13, 15)),
    frozenset((1, 4, 7, 9, 12, 14, 15)),
    frozenset((1, 4, 7, 9, 12, 14, 15)),
    frozenset((1, 4, 7, 9, 12, 14, 15)),
)


def _emit(tc, nc, q_d, k_d, v_d, mk_d, mq_d, o_d):
    ctx_pools = []

    consts = tc.alloc_tile_pool(name="consts", bufs=1)
    sb = tc.alloc_tile_pool(name="sb", bufs=1)
    dscr = tc.alloc_tile_pool(name="dscr", bufs=1, space="DRAM")
    expp = tc.alloc_tile_pool(name="expp", bufs=3)
    otp = tc.alloc_tile_pool(name="otp", bufs=2)
    finp = tc.alloc_tile_pool(name="finp", bufs=2)
    ctx_pools += [consts, sb, dscr, expp, otp, finp]

    identity = consts.tile([128, 128], F32, name="identity")
    make_identity(nc, identity)
    # warm the ACT exp table before the pipeline needs it
    actwarm = consts.tile([1, 1], F32, name="actwarm")
    nc.scalar.activation(out=actwarm, in_=identity[0:1, 0:1], func=Exp)
    bshift = consts.tile([128, 1], F32, name="bshift")
    nc.gpsimd.memset(bshift[:, :], -C_SHIFT)
    identb = consts.tile([128, 128], BF16, name="identb")
    nc.vector.tensor_copy(identb[:, :], identity[:, :])

    q3 = sb.tile([128, NQT, D], F32, name="q3")         # q3[p,n] = q row p*16+n
    k3 = sb.tile([128, NPAIR, 2, D], F32, name="k3")    # k3[p,i,j] = k row p*32+2i+j
    v3f = sb.tile([128, NKB, D], F32, name="v3f")
    mk = sb.tile([128, NKB], F32, name="mk_sb")
    mq = sb.tile([128, NQT], F32, name="mq_sb")
    s1 = sb.tile([128, NQT], F32, name="s1_sb")         # 0.125*(1-mq)
    qb3 = sb.tile([128, NQT, D], F32, name="qb3")       # qs*(1-mq) passthrough
    # Staged tensors are split into per-stage tiles: the Tile framework
    # tracks hazards per tile, so a single big tile would serialize early
    # consumers behind late producers.
    q3bd0 = sb.tile([128, 4, 2, D], BF16, name="q3bd0")   # bf16 q, duplicated
    q3bdR = sb.tile([128, NQT - 4, 2, D], BF16, name="q3bdR")
    k3b_a = sb.tile([128, 6, 2, D], BF16, name="k3b_a")   # bf16 k, pair layout
    k3b_b = sb.tile([128, 5, 2, D], BF16, name="k3b_b")
    k3b_c = sb.tile([128, 5, 2, D], BF16, name="k3b_c")
    qTd0 = sb.tile([128, 4, 128], BF16, name="qTd0")      # qT on both halves
    qTdR = sb.tile([128, NQT - 4, 128], BF16, name="qTdR")
    kTd_a = sb.tile([128, 6, 128], BF16, name="kTd_a")    # lo=even, hi=odd kb
    kTd_b = sb.tile([128, 5, 128], BF16, name="kTd_b")
    kTd_c = sb.tile([128, 5, 128], BF16, name="kTd_c")
    # [mask*V, mask] per 4 key blocks
    vb8 = [sb.tile([128, 4, D + 1], BF16, name=f"vb{g}") for g in range(NKB // 4)]

    q_scrR = dscr.tile([(NQT - 4) * 128, 128], BF16, name="q_scrR")
    k_scr_b = dscr.tile([5 * 128, 128], BF16, name="k_scr_b")
    k_scr_c = dscr.tile([5 * 128, 128], BF16, name="k_scr_c")

    def kt_sel(i):
        if i < 6:
            return kTd_a, i
        if i < 11:
            return kTd_b, i - 6
        return kTd_c, i - 11

    def vb_sel(kb):
        return vb8[kb // 4][:, kb % 4, :]

    qap = q_d.ap().rearrange("(p n) d -> p n d", p=128)
    kap = k_d.ap().rearrange("(p n) d -> p n d", p=128)
    vap = v_d.ap().rearrange("(p n) d -> p n d", p=128)
    oap = o_d.ap().rearrange("(p n) d -> p n d", p=128)

    def stage(eng, scr, b4):
        eng.dma_start(
            out=scr[:, :].rearrange("(i p) c -> p i c", p=128),
            in_=b4[:, :, :, :].rearrange("p i j d -> p i (j d)"))

    def xpose(eng, td, scr):
        eng.dma_start_transpose(
            td[:, :, :].rearrange("p i c -> p (i c)"), scr[:, :])

    # ---- startup loads on the SP ring; bulk via GPSIMD SWDGE ----
    nc.sync.dma_start(out=mk, in_=mk_d.ap().rearrange("(p n) -> p n", p=128))
    nc.sync.dma_start(out=mq, in_=mq_d.ap().rearrange("(p n) -> p n", p=128))
    nc.sync.dma_start(out=q3[:, 0:4, :], in_=qap[:, 0:4, :])
    nc.sync.dma_start(out=k3[:, 0:6, :, :], in_=kap[:, 0:12, :])
    nc.gpsimd.dma_start(out=v3f[:, 0:4, :], in_=vap[:, 0:4, :])

    # Startup transposes on the PE itself: each [128, 128] pair slice is
    # transposed in one shot (even block lands on rows 0:64, odd on rows
    # 64:128, exactly the kTd layout; q uses a duplicated bf16 source).
    # This doubles as the PE p-state warmup, and qTd0/kTd_a become ready
    # several us before any DMA-transpose chain could deliver them.
    nc.gpsimd.tensor_copy(q3bd0[:, :, 0, :], q3[:, 0:4, :])
    nc.gpsimd.tensor_copy(q3bd0[:, :, 1, :], q3[:, 0:4, :])
    nc.vector.tensor_copy(k3b_a, k3[:, 0:6, :, :])
    prep_ps = tc.alloc_tile_pool(name="prep_ps", bufs=1, space="PSUM")
    tq = prep_ps.tile([128, 4, 128], BF16, name="tq")
    tks = [prep_ps.tile([128, 2, 128], BF16, name=f"tk{j}") for j in range(3)]
    for t in range(4):
        nc.tensor.transpose(tq[:, t, :], q3bd0[:, t, :, :], identb)
    nc.vector.tensor_copy(qTd0[:, :, :], tq)
    for j in range(3):
        for i in (2 * j, 2 * j + 1):
            nc.tensor.transpose(tks[j][:, i - 2 * j, :], k3b_a[:, i, :, :], identb)
        nc.scalar.copy(out=kTd_a[:, 2 * j:2 * j + 2, :], in_=tks[j])
    prep_ps.release()

    nc.gpsimd.dma_start(out=v3f[:, 4:8, :], in_=vap[:, 4:8, :])
    for g in range(NKB // 4):
        nc.gpsimd.tensor_copy(vb8[g][:, :, D:D + 1],
                              mk[:, 4 * g:4 * g + 4].rearrange("p (n o) -> p n o", o=1))
    for kb in range(4):
        nc.gpsimd.tensor_scalar_mul(vb_sel(kb)[:, 0:D], v3f[:, kb, :], mk[:, kb:kb + 1])
    nc.gpsimd.dma_start(out=k3[:, 6:NPAIR, :, :], in_=kap[:, 12:NKB, :])
    for kb in range(4, 8):
        nc.gpsimd.tensor_scalar_mul(vb_sel(kb)[:, 0:D], v3f[:, kb, :], mk[:, kb:kb + 1])
    nc.gpsimd.dma_start(out=v3f[:, 8:20, :], in_=vap[:, 8:20, :])
    nc.gpsimd.dma_start(out=q3[:, 4:NQT, :], in_=qap[:, 4:NQT, :])

    # ---- remaining converts (DVE) + stages + transposes, pair-ordered ----
    nc.vector.tensor_copy(k3b_b, k3[:, 6:11, :, :])
    stage(nc.sync, k_scr_b, k3b_b)
    xpose(nc.sync, kTd_b, k_scr_b)
    for kb in range(8, 16):
        nc.gpsimd.tensor_scalar_mul(vb_sel(kb)[:, 0:D], v3f[:, kb, :], mk[:, kb:kb + 1])
    nc.gpsimd.dma_start(out=v3f[:, 20:NKB, :], in_=vap[:, 20:NKB, :])
    nc.vector.tensor_copy(k3b_c, k3[:, 11:NPAIR, :, :])
    stage(nc.sync, k_scr_c, k3b_c)
    xpose(nc.sync, kTd_c, k_scr_c)
    nc.vector.tensor_copy(q3bdR[:, :, 0, :], q3[:, 4:NQT, :])
    nc.vector.tensor_copy(q3bdR[:, :, 1, :], q3[:, 4:NQT, :])
    stage(nc.sync, q_scrR, q3bdR)
    xpose(nc.sync, qTdR, q_scrR)

    # rest of PV weights + passthrough term on GPSIMD, pair-ordered
    for kb in range(16, NKB):
        nc.gpsimd.tensor_scalar_mul(vb_sel(kb)[:, 0:D], v3f[:, kb, :], mk[:, kb:kb + 1])
    nc.gpsimd.tensor_scalar(s1, mq, -0.125, 0.125, MUL, ADD)
    for qt in range(NQT):
        nc.gpsimd.tensor_scalar_mul(qb3[:, qt, :], q3[:, qt, :], s1[:, qt:qt + 1])

    # ---- main loop ----
    ps_sc = tc.alloc_tile_pool(name="ps_sc", bufs=3, space="PSUM")
    ps_o = tc.alloc_tile_pool(name="ps_o", bufs=1, space="PSUM")
    ps_e = tc.alloc_tile_pool(name="ps_e", bufs=1, space="PSUM")
    ctx_pools += [ps_sc, ps_o, ps_e]

    def qk(qc, i):
        kta, il = kt_sel(i)
        qta, q0 = (qTd0, 0) if qc == 0 else (qTdR, 4 * (qc - 1))
        sc = ps_sc.tile([128, 2, QCH], F32, name=f"sc{qc}_{i}", tag="sc")
        nc.tensor.matmul(sc[:, 0, :], lhsT=kta[0:64, il, :],
                         rhs=qta[0:64, q0:q0 + 4, :],
                         start=True, stop=True, tile_position=(0, 0))
        nc.tensor.matmul(sc[:, 1, :], lhsT=kta[64:128, il, :],
                         rhs=qta[64:128, q0:q0 + 4, :],
                         start=True, stop=True, tile_position=(64, 0))
        return sc

    def emit_epilogue(qc, oT_ps):
        oT_sb = otp.tile([D + 1, QCH], F32, name=f"oT_sb{qc}", tag="otsb")
        nc.scalar.copy(out=oT_sb, in_=oT_ps)
        tp = ps_e.tile([128, TPC, D + 1], F32, name=f"tp{qc}", tag="tp")
        for t in range(TPC):
            nc.tensor.transpose(tp[:, t, :], oT_sb[:, 128 * t:128 * (t + 1)],
                                identity[0:D + 1, 0:D + 1])
        rec = finp.tile([128, TPC], F32, name=f"rec{qc}", tag="rec")
        nc.vector.reciprocal(rec, tp[:, :, D:D + 1])
        recm = finp.tile([128, TPC], F32, name=f"recm{qc}", tag="recm")
        nc.vector.tensor_tensor(recm, rec, mq[:, 4 * qc:4 * qc + 4], MUL)
        fin3 = finp.tile([128, TPC, D], F32, name=f"fin3_{qc}", tag="fin3")
        for t in range(TPC):
            nc.vector.scalar_tensor_tensor(fin3[:, t, :], tp[:, t, 0:D],
                                           recm[:, t:t + 1], qb3[:, 4 * qc + t, :],
                                           MUL, ADD)
        nc.sync.dma_start(out=oap[:, 4 * qc:4 * qc + 4, :], in_=fin3)

    pend_epi = None
    for qc in range(NQC):
        oT_ps = ps_o.tile([D + 1, QCH], F32, name=f"oT_ps{qc}", tag="ot")
        scs = {i: qk(qc, i) for i in range(3)}
        if pend_epi is not None:
            emit_epilogue(*pend_epi)
            pend_epi = None
        for i in range(NPAIR):
            sc = scs.pop(i)
            ex = expp.tile([128, 2, QCH], BF16, name=f"ex{qc}_{i}", tag="ex")
            if i in D_PAIRS[qc]:
                nc.vector.tensor_scalar(ex[:, :, :].bitcast(I16), sc[:, :, :],
                                        A_SCHR, B_SCHR, MUL, ADD)
            else:
                nc.scalar.activation(out=ex, in_=sc[:, :, :], func=Exp,
                                     scale=0.125, bias=bshift[:, 0:1])
            if i + 3 < NPAIR:
                scs[i + 3] = qk(qc, i + 3)
            nc.tensor.matmul(oT_ps, lhsT=vb_sel(2 * i), rhs=ex[:, 0, :],
                             start=(i == 0), stop=False)
            nc.tensor.matmul(oT_ps, lhsT=vb_sel(2 * i + 1), rhs=ex[:, 1, :],
                             start=False, stop=(i == NPAIR - 1))
        pend_epi = (qc, oT_ps)
    emit_epilogue(*pend_epi)

    for p in reversed(ctx_pools):
        p.release()


_PROGS = {}


def _build(repeat=1, loop=None, ablate=()):
    key = (repeat, loop, tuple(ablate))
    if key in _PROGS:
        return _PROGS[key]
    nc = bacc.Bacc("TRN2", target_bir_lowering=False, debug=False)
    q_d = nc.dram_tensor("q_in", [QSH, D], F32, kind="ExternalInput")
    k_d = nc.dram_tensor("k_in", [S, D], F32, kind="ExternalInput")
    v_d = nc.dram_tensor("v_in", [S, D], F32, kind="ExternalInput")
    mk_d = nc.dram_tensor("mk_in", [S], F32, kind="ExternalInput")
    mq_d = nc.dram_tensor("mq_in", [QSH], F32, kind="ExternalInput")
    o_d = nc.dram_tensor("o_out", [QSH, D], F32, kind="ExternalOutput")
    with tile.TileContext(nc) as tc:
        if loop is not None:
            with tc.For_i(0, loop, 1):
                for _ in range(repeat):
                    _emit(tc, nc, q_d, k_d, v_d, mk_d, mq_d, o_d)
        else:
            for _ in range(repeat):
                _emit(tc, nc, q_d, k_d, v_d, mk_d, mq_d, o_d)
    nc.compile()
    _PROGS[key] = nc
    return nc


def make_in_maps(q, k, v, mask):
    q = np.ascontiguousarray(np.asarray(q, dtype=np.float32))
    k = np.ascontiguousarray(np.asarray(k, dtype=np.float32))
    v = np.ascontiguousarray(np.asarray(v, dtype=np.float32))
    mask = np.ascontiguousarray(np.asarray(mask, dtype=np.float32))
    in_maps = []
    for c in range(NCORES):
        b, h = c // 2, c % 2
        sl = slice(h * QSH, (h + 1) * QSH)
        in_maps.append({
            "q_in": np.ascontiguousarray(q[b, sl, :]),
            "k_in": np.ascontiguousarray(k[b]),
            "v_in": np.ascontiguousarray(v[b]),
            "mk_in": np.ascontiguousarray(mask[b]),
            "mq_in": np.ascontiguousarray(mask[b, sl]),
        })
    return in_maps


def gather(results):
    out = np.empty((B, S, D), np.float32)
    for c in range(NCORES):
        b, h = c // 2, c % 2
        out[b, h * QSH:(h + 1) * QSH, :] = results[c]["o_out"]
    return out


def kernel(q, k, v, mask, _spmd_kwargs=None):
    nc = _build()
    in_maps = make_in_maps(q, k, v, mask)
    res = run_bass_kernel_spmd(nc, in_maps, core_ids=list(range(NCORES)),
                               **(_spmd_kwargs or {}))
    out = gather(res.results)
    if _spmd_kwargs:
        kernel._last_results = res
    return out
